# revision 1
# baseline (speedup 1.0000x reference)
"""CapsuleLayer dynamic-routing kernel for 8 Trainium2 NeuronCores.

Math (reference):
    u_hat[b,n,j,d] = sum_i W[n,j,d,i] * x[b,j,i]
    b = 0; for r in 0..2:
        c = softmax_n(b); s[b,n,d] = sum_j c*u_hat; v = squash_d(s)
        if r < 2: b += sum_d v*u_hat
    return v  [B, N, D]

Sharding: J (input capsules, 2048) split 8 ways -> Jc=256 per core.
Softmax over n is local; only s needs a 256 KiB AllReduce per iteration.

Per-core dataflow, one sweep over W per routing iteration (u_hat is
recomputed from SBUF-streamed W each iteration; never materialized):
  - j processed in groups of 4: 4 PE sub-matmuls via column tiling
    (tile_position=(0,32r)) produce u_hat group tile
    [128=(4j x 32b), (n,d)] in PSUM.
  - r0: softmax(0) is uniform, so u_hat is accumulated over all j
    directly in PSUM; s0 = (1/N) * strip-sum. No vector work at all.
  - r>=1: logits[p=(j,b), (g,n)] += sum_d v_{r-1}*u_hat  (DVE mult +
    segmented reduce over d); softmax over n is local to each
    (partition, group) -> c; tmp2 = c (x) u_hat on GpSimd.
  - s accumulated over j by a PE matmul with a stacked-identity lhsT
    (sums the 4 j-strips per b), accumulating across groups in PSUM.
    s-matmuls are emitted one group late so they don't block the next
    group's u_hat matmuls in the in-order PE queue.
  - AllReduce s across cores, squash redundantly on every core.
"""

import functools
import numpy as np

B, J, I = 32, 2048, 16
N, D = 64, 32
NCORES = 8
JC = J // NCORES          # 256 j per core
GRP = 4                   # j's per group (PE column strips)
NG = JC // GRP            # 64 groups
ND = N * D                # 2048
HALF = ND // 2            # 1024 free-dim half (PSUM sizing)
NH = N // 2               # 32 n per half
ROUTINGS = 3
EPS = 1e-7


@functools.lru_cache(maxsize=1)
def _build():
    import concourse.bass as bass
    import concourse.mybir as mybir
    import concourse.bacc as bacc
    import concourse.tile as tile

    f32 = mybir.dt.float32
    bf16 = mybir.dt.bfloat16
    MUL = mybir.AluOpType.mult
    ADD = mybir.AluOpType.add
    AX = mybir.AxisListType.X
    AF = mybir.ActivationFunctionType

    nc = bacc.Bacc("TRN2", target_bir_lowering=False, debug=False,
                   num_devices=NCORES)

    xt_d = nc.dram_tensor("xt", [I, JC * B], bf16, kind="ExternalInput")
    wt_d = nc.dram_tensor("wt", [I, JC, ND], bf16, kind="ExternalInput")
    ones_d = nc.dram_tensor("ones4", [GRP * B, B], bf16, kind="ExternalInput")
    v_d = nc.dram_tensor("v", [B, ND], f32, kind="ExternalOutput")

    with tile.TileContext(nc) as tc:
        with (
            tc.tile_pool(name="persist", bufs=1) as pp,
            tc.tile_pool(name="wstream", bufs=4) as wp,
            tc.tile_pool(name="work", bufs=4) as wk,
            tc.tile_pool(name="small", bufs=6) as sm,
            tc.tile_pool(name="ups", bufs=3, space="PSUM") as ups_pool,
            tc.tile_pool(name="sps", bufs=1, space="PSUM") as sps_pool,
            tc.tile_pool(name="dram", bufs=1, space="DRAM") as dr,
        ):
            xt = pp.tile([I, JC * B], bf16)
            nc.sync.dma_start(xt[:], xt_d[:])
            ones4 = pp.tile([GRP * B, B], bf16)
            nc.sync.dma_start(ones4[:], ones_d[:])

            logits = pp.tile([128, NG, N], bf16)
            v_rep = pp.tile([128, N, D], bf16)
            v_small = pp.tile([B, ND], bf16)
            s_sb = pp.tile([128, 512], f32)
            v_sb = pp.tile([B, ND], f32)

            cc_in = dr.tile([128, 512], f32)
            cc_out = dr.tile([128, 512], f32)

            def u_mms(u_ps, w_t, g, h, start, stop):
                """16 col-tiled matmuls for one (group, half); rr-outer so
                consecutive chunk matmuls share the stationary lhsT."""
                for rr in range(GRP):
                    j = g * GRP + rr
                    for cch in range(2):
                        nc.tensor.matmul(
                            u_ps[32 * rr:32 * rr + 32,
                                 cch * 512:(cch + 1) * 512],
                            xt[:, j * B:(j + 1) * B],
                            w_t[:, rr, h * HALF + cch * 512:
                                h * HALF + (cch + 1) * 512],
                            start=start, stop=stop,
                            tile_position=(0, 32 * rr),
                            skip_group_check=True,
                        )

            for r in range(ROUTINGS):
                s_ps = sps_pool.tile([128, 512], f32)

                if r == 0:
                    # -- r0: c is uniform; accumulate u_hat over j in PSUM --
                    acc = [ups_pool.tile([128, HALF], f32, name=f"acc{_h}", tag="u_ps") for _h in range(2)]
                    for g in range(NG):
                        w_t = wp.tile([I, GRP, ND], bf16)
                        nc.sync.dma_start(
                            w_t[:], wt_d[:, g * GRP:(g + 1) * GRP, :])
                        for h in range(2):
                            u_mms(acc[h], w_t, g, h,
                                  start=(g == 0), stop=(g == NG - 1))
                    # evac to bf16 SBUF, then strip-sum via ones4 matmul
                    for h in range(2):
                        a_sb = wk.tile([128, HALF], bf16)
                        nc.scalar.activation(a_sb[:], acc[h][:], AF.Copy)
                        for cch in range(2):
                            q = 2 * h + cch
                            nc.tensor.matmul(
                                s_ps[32 * q:32 * q + 32, :],
                                ones4[:],
                                a_sb[:, cch * 512:(cch + 1) * 512],
                                start=True, stop=True,
                                tile_position=(0, 32 * q),
                                skip_group_check=True,
                            )
                else:
                    # -- r>=1: fused logits update + local softmax + s --
                    pending_smm = []
                    for g in range(NG):
                        w_t = wp.tile([I, GRP, ND], bf16)
                        nc.sync.dma_start(
                            w_t[:], wt_d[:, g * GRP:(g + 1) * GRP, :])

                        c_t = sm.tile([128, N], bf16)
                        zrec = sm.tile([128, 1], f32)

                        u_sb_halves = []
                        for h in range(2):
                            u_ps = ups_pool.tile([128, HALF], f32)
                            u_mms(u_ps, w_t, g, h, start=True, stop=True)
                            # emit previous group's s-matmuls behind this
                            # group's u-matmuls in the PE stream
                            if pending_smm:
                                pending_smm.pop(0)()

                            u_sb = wk.tile([128, NH, D], bf16)
                            nc.scalar.activation(u_sb[:], u_ps[:], AF.Copy)
                            tl = wk.tile([128, NH, D], bf16)
                            nc.vector.tensor_tensor(
                                tl[:], u_sb[:],
                                v_rep[:, h * NH:(h + 1) * NH, :], op=MUL)
                            with nc.allow_low_precision("bf16 routing logits"):
                                if r == 1:
                                    nc.vector.tensor_reduce(
                                        logits[:, g, h * NH:(h + 1) * NH],
                                        tl[:], axis=AX, op=ADD)
                                else:
                                    dtmp = sm.tile([128, NH], bf16)
                                    nc.vector.tensor_reduce(
                                        dtmp[:], tl[:], axis=AX, op=ADD)
                                    nc.vector.tensor_add(
                                        logits[:, g, h * NH:(h + 1) * NH],
                                        logits[:, g, h * NH:(h + 1) * NH],
                                        dtmp[:])
                            u_sb_halves.append(u_sb)

                        # local softmax over n for this group's 4 j's
                        e_t = sm.tile([128, N], f32)
                        nc.scalar.activation(e_t[:], logits[:, g, :], AF.Exp)
                        zsum = sm.tile([128, 1], f32)
                        nc.vector.tensor_reduce(zsum[:], e_t[:], axis=AX, op=ADD)
                        nc.vector.reciprocal(zrec[:], zsum[:])
                        nc.vector.tensor_scalar_mul(c_t[:], e_t[:], zrec[:])

                        # tmp2 = c (x) u_hat on GpSimd (c broadcast over d)
                        t2s = []
                        for h in range(2):
                            t2 = wk.tile([128, NH, D], bf16, name="t2", tag="t2")
                            eng = nc.vector if h == 0 else nc.gpsimd
                            eng.tensor_tensor(
                                t2[:], u_sb_halves[h][:],
                                c_t[:, h * NH:(h + 1) * NH, None]
                                .broadcast_to([128, NH, D]),
                                op=MUL)
                            t2s.append(t2)

                        def make_smm(t2s=t2s, g=g):
                            def emit():
                                for h in range(2):
                                    t2f = t2s[h][:].rearrange("p a b -> p (a b)")
                                    for cch in range(2):
                                        q = 2 * h + cch
                                        nc.tensor.matmul(
                                            s_ps[32 * q:32 * q + 32, :],
                                            ones4[:],
                                            t2f[:, cch * 512:(cch + 1) * 512],
                                            start=(g == 0), stop=(g == NG - 1),
                                            tile_position=(0, 32 * q),
                                            skip_group_check=True,
                                        )
                            return emit
                        pending_smm.append(make_smm())
                    while pending_smm:
                        pending_smm.pop(0)()

                # ---- end of sweep: AllReduce s, squash, update v ----
                # everything below stays in the (quarter, b)-strip layout:
                # partition 32q+b holds n in [16q,16q+16), all of d.
                s_evac = sm.tile([128, 512], f32)
                if r == 0:
                    nc.vector.tensor_scalar_mul(s_evac[:], s_ps[:], 1.0 / N)
                else:
                    nc.vector.tensor_copy(s_evac[:], s_ps[:])
                nc.sync.dma_start(cc_in[:], s_evac[:])
                nc.gpsimd.collective_compute(
                    "AllReduce", ADD,
                    replica_groups=[list(range(NCORES))],
                    ins=[cc_in[:].opt()], outs=[cc_out[:].opt()],
                )
                nc.sync.dma_start(s_sb[:], cc_out[:])

                sq = sm.tile([128, 16, D], f32)
                s3 = s_sb[:].rearrange("p (n d) -> p n d", d=D)
                nc.vector.tensor_tensor(sq[:], s3, s3, op=MUL)
                ns2 = sm.tile([128, 16], f32)
                nc.vector.tensor_reduce(ns2[:], sq[:], axis=AX, op=ADD)
                onep = sm.tile([128, 16], f32)
                nc.vector.tensor_scalar_add(onep[:], ns2[:], 1.0)
                rt = sm.tile([128, 16], f32)
                eps_t = sm.tile([128, 1], f32)
                nc.vector.memset(eps_t[:], EPS)
                nc.scalar.activation(rt[:], ns2[:], AF.Sqrt, bias=eps_t[:])
                den = sm.tile([128, 16], f32)
                nc.vector.tensor_tensor(den[:], onep[:], rt[:], op=MUL)
                dinv = sm.tile([128, 16], f32)
                nc.vector.reciprocal(dinv[:], den[:])
                scl = sm.tile([128, 16], f32)
                nc.vector.tensor_tensor(scl[:], ns2[:], dinv[:], op=MUL)
                v4 = sm.tile([128, 16, D], f32)
                nc.vector.tensor_tensor(
                    v4[:], s3,
                    scl[:, :, None].broadcast_to([128, 16, D]),
                    op=MUL)

                if r < ROUTINGS - 1:
                    v4b = sm.tile([128, 512], bf16)
                    nc.vector.tensor_copy(
                        v4b[:], v4[:].rearrange("p a b -> p (a b)"))
                    for q in range(4):
                        nc.sync.dma_start(
                            v_small[:, q * 512:(q + 1) * 512],
                            v4b[32 * q:32 * q + 32, :])
                    for rr in range(GRP):
                        nc.sync.dma_start(
                            v_rep[32 * rr:32 * rr + 32, :, :],
                            v_small[:].rearrange("b (n d) -> b n d", d=D))
                else:
                    for q in range(4):
                        nc.sync.dma_start(
                            v_sb[:, q * 512:(q + 1) * 512],
                            v4[32 * q:32 * q + 32, :])

            nc.sync.dma_start(v_d[:], v_sb[:])

    nc.compile()
    return nc


def kernel(x: np.ndarray, W: np.ndarray) -> np.ndarray:
    import ml_dtypes
    from concourse.bass_utils import run_bass_kernel_spmd

    nc = _build()

    bf = ml_dtypes.bfloat16
    xt = np.ascontiguousarray(x.transpose(2, 1, 0)).astype(bf)          # [I,J,B]
    wt = np.ascontiguousarray(W.transpose(3, 1, 0, 2).reshape(I, J, ND)).astype(bf)
    ones4 = np.tile(np.eye(B, dtype=np.float32), (GRP, 1)).astype(bf)

    in_maps = []
    for k in range(NCORES):
        jlo, jhi = k * JC, (k + 1) * JC
        in_maps.append({
            "xt": np.ascontiguousarray(xt[:, jlo:jhi, :]).reshape(I, JC * B),
            "wt": np.ascontiguousarray(wt[:, jlo:jhi, :]),
            "ones4": ones4,
        })

    res = run_bass_kernel_spmd(nc, in_maps, list(range(NCORES)))
    v = np.asarray(res.results[0]["v"], dtype=np.float32)
    return v.reshape(B, N, D)


if __name__ == "__main__":
    rng = np.random.default_rng(0)
    x = rng.normal(size=(B, J, I)).astype(np.float32)
    W = rng.normal(size=(N, J, D, I)).astype(np.float32) * 0.05
    v = kernel(x, W)
    print(v.shape, v.dtype, np.abs(v).max())



# revision 15
# speedup vs baseline: 1.4163x; 1.4163x over previous
"""CapsuleLayer dynamic-routing kernel for 8 Trainium2 NeuronCores, v2.

Math (reference):
    u_hat[b,n,j,d] = sum_i W[n,j,d,i] * x[b,j,i]
    b = 0; for r in 0..2:
        c = softmax_n(b); s[b,n,d] = sum_j c*u_hat; v = squash_d(s)
        if r < 2: b += sum_d v*u_hat
    return v  [B, N, D]

Design: u_hat is NEVER materialized (its per-j matmul only uses K=16 of
the 128 PE rows). All big PE work runs at K=128:

  r0:   s0[b,(n,d)] = (1/N) sum_{(j,i)} x[b,(j,i)] W[(j,i),(n,d)]
        -> dense matmuls, stationary x K-tiles [(8j,16i),b].
  r>=1: dlog[b,n,j] = sum_i A[b,n,j,i] x[b,j,i],
        A[b,n,j,i] = sum_d v[b,n,d] W[n,j,d,i] computed by PE with a
        block-diagonal stationary blockdiag(v_n^T for 4 n) [128,128]
        and moving W_A[(q,d),(j,i)] -> out [(q,b),(j,i)] at K=128.
        The i-reduction is a DVE mult + add-tree.
  r>=1: s via y = c (x) x folded: stationary W_S[(j),(c8:q,d)] chunks,
        moving y^T[(j),(q,b)] computed directly in j-partition layout
        from a PE-transposed c (so the 8.4M-element y itself is never
        transposed). Diagonal 4x4 blocks of the [128,128] outputs are
        the s partials, accumulated in PSUM over (i, jt).

Sharding: J (2048) split 8 ways -> Jc=256/core. softmax over n is
local; s is AllReduced (f32, chunked so AR overlaps compute).
Squash runs redundantly per core in the [(q,d),(ch,b)] layout using
tiny PE partition-reduction matmuls; the last iteration transposes to
b-major and squashes on the free dim for the f32 output.
"""

import functools
import numpy as np

B, J, I = 32, 2048, 16
N, D = 64, 32
NCORES = 8
JC = J // NCORES          # 256 j per core
ND = N * D                # 2048
NCH = 16                  # n-blocks of 4 (ch); n = 4*ch + q
ROUTINGS = 3
EPS = 1e-7


@functools.lru_cache(maxsize=2)
def _build(dbg=False):
    import concourse.bass as bass
    import concourse.mybir as mybir
    import concourse.bacc as bacc
    import concourse.tile as tile

    f32 = mybir.dt.float32
    bf16 = mybir.dt.bfloat16
    MUL = mybir.AluOpType.mult
    ADD = mybir.AluOpType.add
    AX = mybir.AxisListType.X
    AF = mybir.ActivationFunctionType

    nc = bacc.Bacc("TRN2", target_bir_lowering=False, debug=False,
                   num_devices=NCORES)

    # --- DRAM inputs (per-core shard layouts, prepared host-side) ---
    wm_d = nc.dram_tensor("wm", [4, 32, 128, 512], bf16, kind="ExternalInput")
    xm_d = nc.dram_tensor("xm", [128, 32, B], bf16, kind="ExternalInput")
    wa_d = nc.dram_tensor("wa", [NCH, 128, JC * I], bf16, kind="ExternalInput")
    xa_d = nc.dram_tensor("xa", [128, JC * I], bf16, kind="ExternalInput")
    ws_d = nc.dram_tensor("ws", [2, I, 2, 128, 8 * 128], bf16,
                          kind="ExternalInput")
    xt_d = nc.dram_tensor("xt", [128, 2, I, B], bf16, kind="ExternalInput")
    id_d = nc.dram_tensor("idm", [128, 128], bf16, kind="ExternalInput")
    idf_d = nc.dram_tensor("idf", [128, 128], f32, kind="ExternalInput")
    sel_d = nc.dram_tensor("sel", [128, B], bf16, kind="ExternalInput")
    rep_d = nc.dram_tensor("rep", [B, 128], bf16, kind="ExternalInput")
    ond_d = nc.dram_tensor("ond", [128, 4], f32, kind="ExternalInput")
    rp4_d = nc.dram_tensor("rp4", [4, 128], f32, kind="ExternalInput")
    v_d = nc.dram_tensor("v", [B, ND], f32, kind="ExternalOutput")
    if dbg:
        dbg_s0 = nc.dram_tensor("dbg_s0", [128, NCH * B], f32,
                                kind="ExternalOutput")
        dbg_lg = nc.dram_tensor("dbg_lg", [128, NCH * 256], bf16,
                                kind="ExternalOutput")
        dbg_c = nc.dram_tensor("dbg_c", [128, NCH * 256], bf16,
                               kind="ExternalOutput")
        dbg_sp = nc.dram_tensor("dbg_sp", [128, NCH * B], f32,
                                kind="ExternalOutput")
        dbg_ct = nc.dram_tensor("dbg_ct", [128, 2 * NCH * 128], bf16,
                                kind="ExternalOutput")
        dbg_y = nc.dram_tensor("dbg_y", [128, 1024], bf16,
                               kind="ExternalOutput")
        dbg_sa = nc.dram_tensor("dbg_sa", [128, 8 * 128], f32,
                                kind="ExternalOutput")

    with tile.TileContext(nc) as tc:
        with (
            tc.tile_pool(name="persist", bufs=1) as pp,
            tc.tile_pool(name="wstream", bufs=3) as wp,
            tc.tile_pool(name="work", bufs=4) as wk,
            tc.tile_pool(name="big1", bufs=1) as bg,
            tc.tile_pool(name="small", bufs=2) as sm,
            tc.tile_pool(name="sqtmp", bufs=1) as sq_p,
            tc.tile_pool(name="psA", bufs=2, space="PSUM") as psA,
            tc.tile_pool(name="psS", bufs=1, space="PSUM") as psS,
            tc.tile_pool(name="psT", bufs=2, space="PSUM") as psT,
            tc.tile_pool(name="dram", bufs=1, space="DRAM") as dr,
        ):
            # ---- persistent tiles ----
            xm = pp.tile([128, 32, B], bf16)
            xa = pp.tile([128, JC * I], bf16)
            xt = pp.tile([128, 2, I, B], bf16)
            idm = pp.tile([128, 128], bf16)
            idf = pp.tile([128, 128], f32)
            sel = pp.tile([128, B], bf16)
            rep = pp.tile([B, 128], bf16)
            ond = pp.tile([128, 4], f32)
            rp4 = pp.tile([4, 128], f32)
            for t_, d_ in ((xm, xm_d), (xa, xa_d), (xt, xt_d), (idm, id_d),
                           (idf, idf_d), (sel, sel_d), (rep, rep_d),
                           (ond, ond_d), (rp4, rp4_d)):
                nc.sync.dma_start(t_[:], d_[:])

            logits = pp.tile([128, NCH, 256], bf16)
            v_bd = pp.tile([128, NCH, 128], bf16)   # blockdiag stationaries
            c_T = pp.tile([128, 2, NCH, 128], bf16)  # [(j), jt, ch, (q,b)]
            s_sb = pp.tile([128, NCH, B], f32)       # [(q,d), ch, b]
            s_full = pp.tile([128, NCH, B], f32)
            eps_t = pp.tile([128, 1], f32)
            nc.vector.memset(eps_t[:], EPS)
            nc.vector.memset(v_bd[:], 0.0)
            nc.vector.memset(logits[:], 0.0)

            def pe_t(out_ps, in_sb, k, ident):
                """PE transpose: out = in_sb[K=k rows, M].T (via identity)."""
                nc.tensor.matmul(out_ps, in_sb, ident[:k, :k],
                                 is_transpose=True, start=True, stop=True,
                                 skip_group_check=True)

            def allreduce(sbuf_in, sbuf_out, cols):
                ci = dr.tile([128, cols], f32)
                co = dr.tile([128, cols], f32)
                nc.sync.dma_start(ci[:], sbuf_in)
                nc.gpsimd.collective_compute(
                    "AllReduce", ADD,
                    replica_groups=[list(range(NCORES))],
                    ins=[ci[:].opt()], outs=[co[:].opt()],
                )
                nc.sync.dma_start(sbuf_out, co[:])

            def squash_update_v():
                """squash in [(q,d),(ch,b)] layout; fill v_bd diagonals."""
                sq = sq_p.tile([128, NCH * B], f32)
                sf = s_full[:].rearrange("p a b -> p (a b)")
                nc.vector.tensor_tensor(sq[:], sf, sf, op=MUL)
                ns2_ps = psT.tile([4, NCH * B], f32, tag="pst")
                nc.tensor.matmul(ns2_ps[:], ond[:], sq[:],
                                 start=True, stop=True, skip_group_check=True)
                ns2 = sq_p.tile([4, NCH * B], f32)
                nc.vector.tensor_copy(ns2[:], ns2_ps[:])
                onep = sq_p.tile([4, NCH * B], f32)
                nc.vector.tensor_scalar_add(onep[:], ns2[:], 1.0)
                rt = sq_p.tile([4, NCH * B], f32)
                nc.scalar.activation(rt[:], ns2[:], AF.Sqrt, bias=eps_t[:4, :])
                den = sq_p.tile([4, NCH * B], f32)
                nc.vector.tensor_tensor(den[:], onep[:], rt[:], op=MUL)
                dinv = sq_p.tile([4, NCH * B], f32)
                nc.vector.reciprocal(dinv[:], den[:])
                scl = sq_p.tile([4, NCH * B], f32)
                nc.vector.tensor_tensor(scl[:], ns2[:], dinv[:], op=MUL)
                sr_ps = psT.tile([128, NCH * B], f32, tag="pst")
                nc.tensor.matmul(sr_ps[:], rp4[:], scl[:],
                                 start=True, stop=True, skip_group_check=True)
                v_sb = sq_p.tile([128, NCH, B], bf16)
                nc.vector.tensor_tensor(
                    v_sb[:].rearrange("p a b -> p (a b)"), sf, sr_ps[:],
                    op=MUL)
                for q in range(4):
                    nc.vector.tensor_copy(
                        v_bd[32 * q:32 * q + 32, :, 32 * q:32 * q + 32],
                        v_sb[32 * q:32 * q + 32, :, :])

            # ================= r0 =================
            # s0 = (1/N) x.T @ W, chunked over cch (512 nd-cols) so each
            # chunk's AllReduce overlaps the next chunk's matmuls.
            for cch in range(4):
                acc = psS.tile([B, 512], f32, tag="pss")
                for t in range(32):
                    wm_t = wp.tile([128, 512], bf16)
                    nc.sync.dma_start(wm_t[:], wm_d[cch, t])
                    nc.tensor.matmul(acc[:], xm[:, t, :], wm_t[:],
                                     start=(t == 0), stop=(t == 31),
                                     skip_group_check=True)
                s0c = sm.tile([B, 512], bf16)
                nc.scalar.activation(s0c[:], acc[:], AF.Copy, scale=1.0 / N)
                for c4 in range(4):
                    ch = 4 * cch + c4
                    tp = psT.tile([128, B], bf16, tag="pst")
                    pe_t(tp[:], s0c[:, 128 * c4:128 * c4 + 128], B, idm)
                    nc.vector.tensor_copy(s_sb[:, ch, :], tp[:])
            allreduce(s_sb[:].rearrange("p a b -> p (a b)"),
                      s_full[:].rearrange("p a b -> p (a b)"), NCH * B)
            if dbg:
                nc.sync.dma_start(dbg_s0[:],
                                  s_full[:].rearrange("p a b -> p (a b)"))
            squash_update_v()

            # ================= r1, r2 =================
            for r in range(1, ROUTINGS):
                # ---- A-phase: logits += sum_i A*x ----
                for ch in range(NCH):
                    wa_t = wp.tile([128, 4096], bf16)
                    nc.sync.dma_start(wa_t[:], wa_d[ch])
                    for qt in range(4):     # quarters of 1024 (64 j x 16 i)
                        aps = psA.tile([128, 1024], f32)
                        for cc in range(2):
                            o = 1024 * qt + 512 * cc
                            nc.tensor.matmul(
                                aps[:, 512 * cc:512 * cc + 512],
                                v_bd[:, ch, :], wa_t[:, o:o + 512],
                                start=True, stop=True, skip_group_check=True)
                        p0 = wk.tile([128, 1024], bf16)
                        nc.scalar.activation(p0[:], aps[:], AF.Copy)
                        p1 = wk.tile([128, 64, 16], bf16)
                        nc.vector.tensor_tensor(
                            p1[:].rearrange("p a b -> p (a b)"), p0[:],
                            xa[:, 1024 * qt:1024 * qt + 1024], op=MUL)
                        t1 = wk.tile([128, 64, 8], bf16)
                        eng = nc.gpsimd if (qt % 2 == 0) else nc.vector
                        eng.tensor_tensor(t1[:], p1[:, :, 0:8], p1[:, :, 8:16],
                                          op=ADD)
                        t2 = sm.tile([128, 64, 4], bf16)
                        nc.vector.tensor_tensor(t2[:], t1[:, :, 0:4],
                                                t1[:, :, 4:8], op=ADD)
                        t3 = sm.tile([128, 64, 2], bf16)
                        nc.vector.tensor_tensor(t3[:], t2[:, :, 0:2],
                                                t2[:, :, 2:4], op=ADD)
                        t4 = sm.tile([128, 64], bf16)
                        nc.vector.tensor_tensor(t4[:], t3[:, :, 0],
                                                t3[:, :, 1], op=ADD)
                        with nc.allow_low_precision("bf16 routing logits"):
                            nc.vector.tensor_tensor(
                                logits[:, ch, 64 * qt:64 * qt + 64],
                                logits[:, ch, 64 * qt:64 * qt + 64],
                                t4[:], op=ADD)

                # ---- softmax over n = (ch, q), local to (b, j) ----
                ee = bg.tile([128, NCH, 256], bf16)
                nc.scalar.activation(ee[:], logits[:], AF.Exp)
                e1 = sm.tile([128, 8, 256], bf16)
                nc.vector.tensor_tensor(e1[:], ee[:, 0:8, :], ee[:, 8:16, :],
                                        op=ADD)
                e2 = sm.tile([128, 4, 256], bf16)
                nc.vector.tensor_tensor(e2[:], e1[:, 0:4, :], e1[:, 4:8, :],
                                        op=ADD)
                e3 = sm.tile([128, 2, 256], bf16)
                nc.vector.tensor_tensor(e3[:], e2[:, 0:2, :], e2[:, 2:4, :],
                                        op=ADD)
                e4 = sm.tile([128, 256], bf16)
                nc.vector.tensor_tensor(e4[:], e3[:, 0, :], e3[:, 1, :],
                                        op=ADD)
                z_ps = psT.tile([B, 256], f32, tag="pst")
                nc.tensor.matmul(z_ps[:], sel[:], e4[:],
                                 start=True, stop=True, skip_group_check=True)
                zrec = sm.tile([B, 256], f32)
                nc.vector.reciprocal(zrec[:], z_ps[:])
                zrecb = sm.tile([B, 256], bf16)
                nc.vector.tensor_copy(zrecb[:], zrec[:])
                zr_ps = psT.tile([128, 256], f32, tag="pst")
                nc.tensor.matmul(zr_ps[:], rep[:], zrecb[:],
                                 start=True, stop=True, skip_group_check=True)
                zr = sm.tile([128, 256], bf16)
                nc.scalar.activation(zr[:], zr_ps[:], AF.Copy)
                cc_t = bg.tile([128, NCH, 256], bf16)
                nc.vector.tensor_tensor(
                    cc_t[:], ee[:],
                    zr[:, None, :].broadcast_to([128, NCH, 256]), op=MUL)
                if dbg and r == 1:
                    nc.sync.dma_start(
                        dbg_lg[:], logits[:].rearrange("p a b -> p (a b)"))
                    nc.sync.dma_start(
                        dbg_c[:], cc_t[:].rearrange("p a b -> p (a b)"))

                # ---- c^T via PE transposes ----
                for ch in range(NCH):
                    for jt in range(2):
                        ctp = psT.tile([128, 128], bf16, tag="pst")
                        pe_t(ctp[:], cc_t[:, ch, 128 * jt:128 * jt + 128],
                             128, idm)
                        nc.scalar.activation(c_T[:, jt, ch, :], ctp[:],
                                             AF.Copy)

                if dbg and r == 1:
                    nc.sync.dma_start(
                        dbg_ct[:],
                        c_T[:].rearrange("p a b c -> p (a b c)"))

                # ---- s-phase: two ch-blocks so AR(chB=0) overlaps ----
                for chB in range(2):
                    sacc = psS.tile([128, 8, 128], f32, tag="pss")
                    nc.vector.memset(sacc[:], 0.0)
                    for i in range(I):
                        for jt in range(2):
                            ws_t = wp.tile([128, 8 * 128], bf16)
                            nc.sync.dma_start(ws_t[:], ws_d[chB, i, jt])
                            y_t = wk.tile([128, 8, 4, B], bf16)
                            nc.vector.tensor_tensor(
                                y_t[:],
                                c_T[:, jt, 8 * chB:8 * chB + 8, :]
                                .rearrange("p c (q b) -> p c q b", b=B),
                                xt[:, jt, i, None, None, :]
                                .broadcast_to([128, 8, 4, B]),
                                op=MUL)
                            if dbg and r == 1 and chB == 0 and i == 0 \
                                    and jt == 0:
                                nc.sync.dma_start(
                                    dbg_y[:],
                                    y_t[:].rearrange("p a b c -> p (a b c)"))
                            last = (i == I - 1 and jt == 1)
                            for c8 in range(8):
                                nc.tensor.matmul(
                                    sacc[:, c8, :],
                                    ws_t[:, 128 * c8:128 * c8 + 128],
                                    y_t[:, c8, :, :]
                                    .rearrange("p q b -> p (q b)"),
                                    start=False, stop=last,
                                    skip_group_check=True)
                    if dbg and r == 1 and chB == 0:
                        pass
                    for c8 in range(8):
                        for q in range(4):
                            nc.vector.tensor_copy(
                                s_sb[32 * q:32 * q + 32, 8 * chB + c8, :],
                                sacc[32 * q:32 * q + 32, c8,
                                     32 * q:32 * q + 32])
                if dbg and r == 1:
                    nc.sync.dma_start(dbg_sp[:],
                                      s_sb[:].rearrange("p a b -> p (a b)"))
                allreduce(s_sb[:].rearrange("p a b -> p (a b)"),
                          s_full[:].rearrange("p a b -> p (a b)"), NCH * B)

                if r < ROUTINGS - 1:
                    squash_update_v()
                else:
                    # final: transpose to b-major, squash on free dim, out
                    v_out = sm.tile([128, 4, 128], f32)
                    for k4 in range(4):
                        stp = psT.tile([128, 128], f32, tag="pst")
                        pe_t(stp[:],
                             s_full[:, 4 * k4:4 * k4 + 4, :]
                             .rearrange("p a b -> p (a b)"), 128, idf)
                        sT = sm.tile([128, 4, D], f32)
                        nc.vector.tensor_copy(
                            sT[:].rearrange("p a b -> p (a b)"), stp[:])
                        sq2 = sm.tile([128, 4, D], f32)
                        nc.vector.tensor_tensor(sq2[:], sT[:], sT[:], op=MUL)
                        ns2 = sq_p.tile([128, 4], f32)
                        nc.vector.tensor_reduce(ns2[:], sq2[:], axis=AX,
                                                op=ADD)
                        onep = sq_p.tile([128, 4], f32)
                        nc.vector.tensor_scalar_add(onep[:], ns2[:], 1.0)
                        rt = sq_p.tile([128, 4], f32)
                        nc.scalar.activation(rt[:], ns2[:], AF.Sqrt,
                                             bias=eps_t[:])
                        den = sq_p.tile([128, 4], f32)
                        nc.vector.tensor_tensor(den[:], onep[:], rt[:],
                                                op=MUL)
                        dinv = sq_p.tile([128, 4], f32)
                        nc.vector.reciprocal(dinv[:], den[:])
                        scl = sq_p.tile([128, 4], f32)
                        nc.vector.tensor_tensor(scl[:], ns2[:], dinv[:],
                                                op=MUL)
                        nc.vector.tensor_tensor(
                            v_out[:, k4, :].rearrange("p (a b) -> p a b", b=D),
                            sT[:],
                            scl[:, :, None].broadcast_to([128, 4, D]), op=MUL)
                    vd = v_d[:].rearrange("b (k f) -> b k f", f=128)
                    for k4 in range(4):
                        for chm in range(4):
                            nc.sync.dma_start(
                                vd[:, 4 * k4 + chm, :],
                                v_out[32 * chm:32 * chm + 32, k4, :])

    nc.compile()
    return nc


def make_in_maps(x: np.ndarray, W: np.ndarray):
    import ml_dtypes
    bf = ml_dtypes.bfloat16
    in_maps = []
    idm = np.eye(128, dtype=np.float32).astype(bf)
    idf = np.eye(128, dtype=np.float32)
    sel = np.tile(np.eye(B, dtype=np.float32), (4, 1)).astype(bf)
    rep = np.tile(np.eye(B, dtype=np.float32), (1, 4)).astype(bf)
    ond = np.kron(np.eye(4, dtype=np.float32), np.ones((32, 1), np.float32))
    rp4 = np.kron(np.eye(4, dtype=np.float32), np.ones((1, 32), np.float32))
    for k in range(NCORES):
        Wk = np.ascontiguousarray(W[:, k * JC:(k + 1) * JC])  # [64,256,32,16]
        xk = np.ascontiguousarray(x[:, k * JC:(k + 1) * JC])  # [32,256,16]
        # wm[cch, t, (8jr,16i), 512] = Wk[n, 8t+jr, d, i], nd=512cch+col
        t1 = Wk.transpose(1, 3, 0, 2).reshape(32, 8, 16, 4, 512)
        wm = np.ascontiguousarray(
            t1.transpose(3, 0, 1, 2, 4).reshape(4, 32, 128, 512))
        # xm[(jr,i), t, b] = xk[b, 8t+jr, i]
        xm = np.ascontiguousarray(
            xk.transpose(1, 2, 0).reshape(32, 8, 16, B)
            .transpose(1, 2, 0, 3).reshape(128, 32, B))
        # wa[ch, (q,d), (j,i)] = Wk[4ch+q, j, d, i]
        wa = np.ascontiguousarray(
            Wk.transpose(0, 2, 1, 3).reshape(NCH, 128, JC * I))
        # xa[(q,b), (j,i)] = xk[b, j, i], replicated over q
        xa = np.ascontiguousarray(np.tile(xk.reshape(B, JC * I), (4, 1)))
        # ws[chB, i, jt, j, (c8,q,d)] = Wk[4*(8chB+c8)+q, 128jt+j, d, i]
        t2 = Wk.transpose(3, 1, 0, 2)            # [i, j, n, d]
        t2 = t2.reshape(16, 2, 128, 2, 8, 128)   # [i, jt, j, chB, c8, (q,d)]
        ws = np.ascontiguousarray(
            t2.transpose(3, 0, 1, 2, 4, 5).reshape(2, 16, 2, 128, 8 * 128))
        # xt[j, jt, i, b] = xk[b, 128jt+j, i]
        xt = np.ascontiguousarray(
            xk.transpose(1, 2, 0).reshape(2, 128, 16, B).transpose(1, 0, 2, 3))
        in_maps.append({
            "wm": wm.astype(bf), "xm": xm.astype(bf),
            "wa": wa.astype(bf), "xa": xa.astype(bf),
            "ws": ws.astype(bf), "xt": xt.astype(bf),
            "idm": idm, "idf": idf, "sel": sel, "rep": rep,
            "ond": ond, "rp4": rp4,
        })
    return in_maps


def kernel(x: np.ndarray, W: np.ndarray) -> np.ndarray:
    from concourse.bass_utils import run_bass_kernel_spmd

    nc = _build()
    in_maps = make_in_maps(x, W)
    res = run_bass_kernel_spmd(nc, in_maps, list(range(NCORES)))
    v = np.asarray(res.results[0]["v"], dtype=np.float32)
    return v.reshape(B, N, D)


if __name__ == "__main__":
    rng = np.random.default_rng(0)
    x = rng.normal(size=(B, J, I)).astype(np.float32)
    W = rng.normal(size=(N, J, D, I)).astype(np.float32) * 0.05
    v = kernel(x, W)
    print(v.shape, v.dtype, np.abs(v).max())
